# revision 24
# baseline (speedup 1.0000x reference)
"""Trainium2 Bass kernel for nn_MemoryBankV2 (memory-bank attention block).

Strategy (v2):
  - batch interleaved across 8 cores (core d gets items d, d+8, ...,
    d+120, rows sorted by item) -> every core sees the same triangular
    visibility structure, so score/Z/attn matmuls restrict their free
    (row) range per memory-chunk pair and skip ~45% of attention work.
  - memory "v projection" reassociated: attn = Wv @ (sum_m e_m mem_m),
    so the per-chunk v-projections + casts disappear; only a 4-bank
    weighted-sum accumulator (u) and one small Wv matmul remain.
  - fp8(e4m3) DoubleRow matmuls for all projections/FFN/gate (weights
    pre-scaled x32 on host, un-scaled in the psum->sbuf casts); bf16
    residual stream; LN stats via bf16 matmuls; Rsqrt/Reciprocal on
    ScalarE instead of DVE reciprocal.
  - gate (last layer) pushed through LN2 algebraically so its matmuls
    run on raw x during the LN2 stats; item-0 rows fixed up on host
    (out == input exactly there).
  - k/q biases dropped (softmax row-shift invariance); v bias folded
    into the attention normalize; HAM warm-up matmuls cover the initial
    DMA wait.
"""

import sys

import numpy as np

sys.path.insert(0, "/opt/trn_rl_repo")

import ml_dtypes  # noqa: E402

import concourse.bass as bass  # noqa: E402
import concourse.mybir as mybir  # noqa: E402
import concourse.tile as tile  # noqa: E402
from concourse import bacc  # noqa: E402
from concourse.bass import ds  # noqa: E402
from concourse.bass_utils import run_bass_kernel_spmd  # noqa: E402

B, T, D, L = 128, 32, 512, 2
NCORES = 8
BLOC = B // NCORES      # 16 items per core
R = BLOC * T            # 512 rows per core
M = B * T               # 4096 memory entries
DT = D // 128           # 4 feature subtiles
FT = (4 * D) // 128     # 16 ffn subtiles
MT = M // 128           # 32 memory chunks
NPAIR = MT // 2         # 16 chunk pairs
NCHUNK = M // 512       # 8 kproj chunks (512 entries each)

F32 = mybir.dt.float32
BF16 = mybir.dt.bfloat16
FP8 = mybir.dt.float8e4
F8NP = ml_dtypes.float8_e4m3
BF = ml_dtypes.bfloat16
AF = mybir.ActivationFunctionType
ALU = mybir.AluOpType
DR = mybir.MatmulPerfMode.DoubleRow

WS = 32.0               # host weight pre-scale before fp8 cast
IVS = 1.0 / WS
SC = 1.0 / float(np.sqrt(D))

# params tensor column layout (per layer l, base = 48*l)
P_BQ, P_BV, P_G1, P_BE1, P_B1, P_B2, P_G2, P_BE2 = 0, 4, 12, 16, 20, 36, 40, 44
P_BS, P_S1, P_C = 96, 100, 104   # globals (gate)
P_COLS = 128


def _row0(p):
    """first row group included for chunk pair p (rows 32*p..512)."""
    return 32 * p


def _build():
    nc = bacc.Bacc("TRN2", target_bir_lowering=False, debug=False)

    memT_d = nc.dram_tensor("memT8", [D, M], FP8, kind="ExternalInput").ap()
    mem8_d = nc.dram_tensor("mem8", [M, D], FP8, kind="ExternalInput").ap()
    xin8_d = nc.dram_tensor("xin8", [D, R], FP8, kind="ExternalInput").ap()
    xbf_d = nc.dram_tensor("x_bf", [D, R], BF16, kind="ExternalInput").ap()
    x0_d = nc.dram_tensor("x0_f32", [D, R], F32, kind="ExternalInput").ap()
    bb_d = nc.dram_tensor("b_bcast", [128, R], BF16, kind="ExternalInput").ap()
    iv_d = nc.dram_tensor("item_vals", [128, MT], BF16, kind="ExternalInput").ap()
    prm_d = nc.dram_tensor("params", [128, P_COLS], F32, kind="ExternalInput").ap()
    wq_d, wk_d, wv_d, w1_d, w2_d = [], [], [], [], []
    for l in range(L):
        wq_d.append(nc.dram_tensor(f"wq{l}", [D, D], FP8, kind="ExternalInput").ap())
        wk_d.append(nc.dram_tensor(f"wk{l}", [D, D], FP8, kind="ExternalInput").ap())
        wv_d.append(nc.dram_tensor(f"wv{l}", [D, D], FP8, kind="ExternalInput").ap())
        w1_d.append(nc.dram_tensor(f"w1{l}", [D, 4 * D], FP8, kind="ExternalInput").ap())
        w2_d.append(nc.dram_tensor(f"w2{l}", [4 * D, D], FP8, kind="ExternalInput").ap())
    ws_d = nc.dram_tensor("ws", [2 * D, D], FP8, kind="ExternalInput").ap()
    out_d = nc.dram_tensor("outT", [D, R], F32, kind="ExternalOutput").ap()

    with tile.TileContext(nc) as tc:
        with (
            tc.tile_pool(name="sb", bufs=1) as sb,
            tc.tile_pool(name="ps", bufs=1, space="PSUM") as ps,
        ):
            # --- constants + HAM warm-up (no DMA dependency) ----------------
            warm = sb.tile([128, 128], BF16, tag="warm", name="warm_sb")
            nc.vector.memset(warm, 0.001)
            wzps = ps.tile([128, 512], F32, tag="z", bufs=1, name="warm_ps")
            for _ in range(52):
                nc.tensor.matmul(wzps[:, 0:64], warm, warm[:, 0:64],
                                 start=True, stop=True)
            ones8 = sb.tile([128, 2, 128], FP8, tag="ones8", name="ones8_sb")
            nc.vector.memset(ones8, 1.0)
            onesw = sb.tile([128, 128], BF16, tag="onesw", name="onesw_sb")
            nc.vector.memset(onesw, 1.0 / D)
            epsln = sb.tile([128, 1], F32, tag="epsln", name="epsln_sb")
            nc.vector.memset(epsln, 1e-5)

            # --- input DMAs (emission order = issue order) ------------------
            # first: what qproj needs, then kproj, then the rest
            def ld_w(l):
                wq = sb.tile([128, DT, D], FP8, tag="wq", bufs=2, name="wq_sb")
                nc.sync.dma_start(out=wq, in_=wq_d[l].rearrange("(a p) n -> p a n", p=128))
                wk = sb.tile([128, DT, D], FP8, tag="wk", bufs=2, name="wk_sb")
                nc.sync.dma_start(out=wk, in_=wk_d[l].rearrange("(a p) n -> p a n", p=128))
                return wq, wk

            def ld_wv(l):
                wv = sb.tile([128, DT, D], FP8, tag="wv", bufs=2, name="wv_sb")
                nc.sync.dma_start(out=wv, in_=wv_d[l].rearrange("(a p) n -> p a n", p=128))
                return wv

            xin8 = sb.tile([128, DT, R], FP8, tag="xin8", name="xin8_sb")
            nc.sync.dma_start(out=xin8, in_=xin8_d.rearrange("(a p) n -> p a n", p=128))
            prm = sb.tile([128, P_COLS], F32, tag="prm", name="prm_sb")
            nc.sync.dma_start(out=prm, in_=prm_d[:, :])
            layer_w = [ld_w(0)]
            memT = sb.tile([128, DT, M], FP8, tag="memT", name="memT_sb")
            for a in range(DT):
                sl = slice(a * 128, (a + 1) * 128)
                nc.sync.dma_start(out=memT[:, a, 0:512], in_=memT_d[sl, 0:512])
            for a in range(DT):
                sl = slice(a * 128, (a + 1) * 128)
                nc.sync.dma_start(out=memT[:, a, 512:M], in_=memT_d[sl, 512:M])
            bb = sb.tile([128, R], BF16, tag="bb", name="bb_sb")
            nc.sync.dma_start(out=bb, in_=bb_d[:, :])
            iv = sb.tile([128, MT], BF16, tag="iv", name="iv_sb")
            nc.sync.dma_start(out=iv, in_=iv_d[:, :])
            layer_wv = [ld_wv(0)]
            mem8 = sb.tile([128, MT, D], FP8, tag="mem8", name="mem8_sb")
            nc.sync.dma_start(out=mem8[:, 0:16, :],
                              in_=mem8_d[0:2048, :].rearrange("(mt p) d -> p mt d", p=128))
            nc.sync.dma_start(out=mem8[:, 16:MT, :],
                              in_=mem8_d[2048:M, :].rearrange("(mt p) d -> p mt d", p=128))
            x = sb.tile([128, DT, R], BF16, tag="x", name="x_sb")
            nc.sync.dma_start(out=x, in_=xbf_d.rearrange("(a p) n -> p a n", p=128))
            x0 = sb.tile([128, DT, R], F32, tag="x0", name="x0_sb")
            nc.sync.dma_start(out=x0, in_=x0_d.rearrange("(a p) n -> p a n", p=128))

            # filler queues: PE work for the next layer, drained into the
            # pointwise (PE-idle) zones of the current layer. kq = layer-1
            # kproj groups (4 per 512-entry chunk, chunk-ascending); gateq =
            # gate x0-half groups.
            kq: list = []
            gateq: list = []
            kpop = [0]

            def drain_filler(n):
                for _ in range(n):
                    if kq:
                        kpop[0] += 1
                        kq.pop(0)()
                    elif gateq:
                        gateq.pop(0)()
                    else:
                        return

            def ensure_k(groups):
                """guarantee the first `groups` layer-1 kproj groups are
                emitted (scores of layer 1 depend on them)."""
                while kq and kpop[0] < groups:
                    kpop[0] += 1
                    kq.pop(0)()

            def dummy_act(func):
                """tiny op to pull the ACT table load off the critical path."""
                dt_ = sb.tile([128, 1], F32, tag="dum", bufs=2, name="dum_sb")
                nc.scalar.activation(out=dt_, in_=epsln, func=func)

            ncast = [0]

            def cast_op(dst, src, scale=IVS, bias=None, func=AF.Identity):
                """psum->sbuf cast, alternating engines to balance load."""
                ncast[0] += 1
                if ncast[0] % 2:
                    nc.scalar.activation(out=dst, in_=src, func=func,
                                         bias=bias if bias is not None else 0.0,
                                         scale=scale)
                else:
                    if bias is None:
                        nc.vector.tensor_scalar(out=dst, in0=src, scalar1=scale,
                                                scalar2=None, op0=ALU.mult)
                    else:
                        nc.vector.tensor_scalar(out=dst, in0=src, scalar1=scale,
                                                scalar2=bias, op0=ALU.mult,
                                                op1=ALU.add)

            def emit_kproj_group(wk, kT, c, j, tag="mm", tagbufs=3):
                kps = ps.tile([128, 512], F32, tag=tag, bufs=tagbufs, name="k_ps")
                for ap_ in range(2):
                    nc.tensor.matmul(kps, wk[:, 2 * ap_:2 * ap_ + 2, ds(j * 128, 128)],
                                     memT[:, 2 * ap_:2 * ap_ + 2, ds(c * 512, 512)],
                                     start=(ap_ == 0), stop=(ap_ == 1), perf_mode=DR)
                cast_op(kT[:, j, ds(c * 512, 512)], kps)

            def _ln_stats(xcur):
                """returns (mups, sqps) psum tiles ([128,R] each, bf16 inputs)."""
                mups = ps.tile([128, R], F32, tag="mm", bufs=3, name="ln_mu")
                for a in range(DT):
                    nc.tensor.matmul(mups, onesw, xcur[:, a, :],
                                     start=(a == 0), stop=(a == DT - 1))
                drain_filler(1)
                sqps = ps.tile([128, R], F32, tag="mm", bufs=3, name="ln_sq")
                for a in range(DT):
                    sq = sb.tile([128, R], BF16, tag="sq", bufs=2, name="ln_sqt")
                    if a % 2:
                        nc.scalar.activation(out=sq, in_=xcur[:, a, :], func=AF.Square)
                    else:
                        nc.vector.tensor_mul(sq, xcur[:, a, :], xcur[:, a, :])
                    nc.tensor.matmul(sqps, onesw, sq,
                                     start=(a == 0), stop=(a == DT - 1))
                drain_filler(2)
                return mups, sqps

            def warm_tick(dep=None):
                """tiny matmul to keep the PE HAM un-throttled through
                pointwise zones (idle >3.4us re-throttles to 1.2GHz). `dep`
                (a just-written bf16 SBUF tile) paces the tick to the
                pointwise chain — without it the in-order PE queue would
                execute all ticks immediately."""
                wt = ps.tile([128, R], F32, tag="mm", bufs=3, name="wt_ps")
                rhs = warm[:, 0:64] if dep is None else dep[:, 0:64]
                nc.tensor.matmul(wt[:, 0:64], warm, rhs, start=True, stop=True)

            def _ln_scalar_chain(mups, sqps):
                """var -> rstd -> murstd; returns (rstd32, rstdb, murstd_bf)."""
                mu2 = sb.tile([128, R], BF16, tag="lns", bufs=2, name="ln_mu2")
                nc.scalar.activation(out=mu2, in_=mups, func=AF.Square)
                var = sb.tile([128, R], BF16, tag="lns", bufs=2, name="ln_var")
                nc.vector.tensor_sub(var, sqps, mu2)
                sd = sb.tile([128, R], F32, tag="lnsd", bufs=2, name="ln_sd")
                nc.scalar.activation(out=sd, in_=var, func=AF.Sqrt,
                                     bias=epsln, scale=1.0)
                rstd32 = sb.tile([128, R], F32, tag="lnr32", bufs=2, name="ln_rstd32")
                nc.vector.reciprocal_approx_fast(out=rstd32, in_=sd)
                warm_tick(var)
                murstd = sb.tile([128, R], BF16, tag="lnr", bufs=2, name="ln_mur")
                nc.vector.tensor_mul(murstd, mups, rstd32)
                rstdb = sb.tile([128, R], BF16, tag="lnr", bufs=2, name="ln_rstd")
                nc.vector.tensor_scalar(out=rstdb, in0=rstd32, scalar1=1.0,
                                        scalar2=None, op0=ALU.mult)
                return rstd32, rstdb, murstd

            def layernorm(xcur, gcol, bcol, out_f8):
                """x <- LN(x) (bf16, in place); fp8 copy to out_f8 (written
                first, all subtiles, so the FFN can start sooner)."""
                dummy_act(AF.Square)
                dummy_act(AF.Sqrt)
                mups, sqps = _ln_stats(xcur)
                _, rstd, murstd = _ln_scalar_chain(mups, sqps)
                us = []
                for a in range(DT):
                    t = sb.tile([128, R], BF16, tag="lnt", bufs=2, name="ln_t")
                    nc.vector.tensor_mul(t, xcur[:, a, :], rstd)
                    u = sb.tile([128, R], BF16, tag="lnu", bufs=4, name="ln_u")
                    nc.vector.tensor_sub(u, t, murstd)
                    nc.vector.tensor_scalar(out=out_f8[:, a, :], in0=u,
                                            scalar1=prm[:, gcol + a:gcol + a + 1],
                                            scalar2=prm[:, bcol + a:bcol + a + 1],
                                            op0=ALU.mult, op1=ALU.add)
                    us.append(u)
                    if a % 2:
                        drain_filler(1)
                        warm_tick(u)
                for a in range(DT):
                    nc.vector.tensor_scalar(out=xcur[:, a, :], in0=us[a],
                                            scalar1=prm[:, gcol + a:gcol + a + 1],
                                            scalar2=prm[:, bcol + a:bcol + a + 1],
                                            op0=ALU.mult, op1=ALU.add)
                    if a % 2:
                        drain_filler(1)
                        warm_tick(us[a])

            xf8_cur = xin8
            for l in range(L):
                base = 48 * l
                wq, wk = layer_w[l]
                wv = layer_wv[l]

                # --- q projection (fp8 out, bias folded into cast) ----------
                qf8 = sb.tile([128, DT, R], FP8, tag="qf8", bufs=1, name="q_sb")
                for j in range(DT):
                    qps = ps.tile([128, R], F32, tag="mm", bufs=3, name="q_ps")
                    for ap_ in range(2):
                        nc.tensor.matmul(qps, wq[:, 2 * ap_:2 * ap_ + 2, ds(j * 128, 128)],
                                         xf8_cur[:, 2 * ap_:2 * ap_ + 2, :],
                                         start=(ap_ == 0), stop=(ap_ == 1), perf_mode=DR)
                    nc.scalar.activation(out=qf8[:, j, :], in_=qps, func=AF.Identity,
                                         bias=prm[:, base + P_BQ + j:base + P_BQ + j + 1],
                                         scale=IVS)

                # --- kT projection ([dout, m] fp8, resident) ----------------
                if l == 0:
                    kT = sb.tile([128, DT, M], FP8, tag="kT", name="kT_sb")
                    for c in range(NCHUNK):
                        for j in range(DT):
                            emit_kproj_group(wk, kT, c, j)
                else:
                    kT = kT_next  # noqa: F821  (built by layer l-1's filler)
                    ensure_k(16)  # chunks 0..3 ready; rest drain in-loop
                    dummy_act(AF.Exp)

                # --- attention: triangular chunk-pair loop ------------------
                ups = ps.tile([128, DT, R], F32, tag="acc4", bufs=1, name="u_ps")
                zps = ps.tile([128, R], F32, tag="z", bufs=1, name="z_ps")
                for p in range(NPAIR):
                    if l == 1:
                        # keep layer-1 kproj fillers 2 chunks ahead of the
                        # scores that consume them (chunk c feeds pairs 2c)
                        ensure_k(4 * min(p // 2 + 3, NCHUNK))
                    ro = _row0(p)
                    n = R - ro
                    e2 = sb.tile([128, 2, R], FP8, tag="e2", bufs=3, name="e2_sb")
                    for c in range(2):
                        mt = 2 * p + c
                        sps = ps.tile([128, R], F32, tag="mm", bufs=3, name="s_ps")
                        for ap_ in range(2):
                            nc.tensor.matmul(
                                sps[:, 0:n],
                                kT[:, 2 * ap_:2 * ap_ + 2, ds(mt * 128, 128)],
                                qf8[:, 2 * ap_:2 * ap_ + 2, ds(ro, n)],
                                start=(ap_ == 0), stop=(ap_ == 1), perf_mode=DR)
                        eraw = sb.tile([128, R], FP8, tag="eraw", bufs=3, name="eraw_sb")
                        nc.scalar.activation(out=eraw[:, 0:n], in_=sps[:, 0:n],
                                             func=AF.Exp, scale=SC)
                        nc.vector.scalar_tensor_tensor(
                            out=e2[:, c, ds(ro, n)], in0=bb[:, ds(ro, n)],
                            scalar=iv[:, mt:mt + 1], in1=eraw[:, 0:n],
                            op0=ALU.is_gt, op1=ALU.mult)
                    nc.tensor.matmul(zps[:, ds(ro, n)], ones8, e2[:, :, ds(ro, n)],
                                     start=(p == 0), stop=(p == NPAIR - 1),
                                     perf_mode=DR, skip_group_check=True)
                    for a in range(DT):
                        nc.tensor.matmul(
                            ups[:, a, ds(ro, n)],
                            mem8[:, 2 * p:2 * p + 2, ds(a * 128, 128)],
                            e2[:, :, ds(ro, n)],
                            start=(p == 0), stop=(p == NPAIR - 1),
                            perf_mode=DR, skip_group_check=True)

                # --- u -> bf16, attn = Wv @ u (x32), normalize+bias+residual
                u8 = sb.tile([128, DT, R], BF16, tag="u8", bufs=1, name="u8_sb")
                for a in range(DT):
                    cast_op(u8[:, a, :], ups[:, a, :], scale=1.0)
                zt = sb.tile([128, R], F32, tag="rz", bufs=2, name="zt_sb")
                nc.vector.tensor_scalar(out=zt, in0=zps, scalar1=WS,
                                        scalar2=WS * 1e-9, op0=ALU.mult,
                                        op1=ALU.add)
                rz = sb.tile([128, R], F32, tag="rz", bufs=2, name="rz_sb")
                nc.vector.reciprocal_approx_fast(out=rz, in_=zt)
                atps = ps.tile([128, DT, R], F32, tag="acc4", bufs=1, name="at_ps")
                for j in range(DT):
                    for a in range(DT):
                        nc.tensor.matmul(atps[:, j, :], wv[:, a, ds(j * 128, 128)],
                                         u8[:, a, :],
                                         start=(a == 0), stop=(a == DT - 1))
                for j in range(DT):
                    at = sb.tile([128, R], BF16, tag="at", bufs=2, name="at_sb")
                    nc.vector.tensor_mul(at, atps[:, j, :], rz)
                    nc.vector.scalar_tensor_tensor(
                        out=x[:, j, :], in0=at,
                        scalar=prm[:, base + P_BV + j:base + P_BV + j + 1],
                        in1=x[:, j, :], op0=ALU.add, op1=ALU.add)
                    drain_filler(1)
                    if j % 2:
                        warm_tick()

                # enqueue next layer's kT projection as filler PE work
                if l + 1 < L:
                    layer_w.append(ld_w(l + 1))
                    layer_wv.append(ld_wv(l + 1))
                    kT_next = sb.tile([128, DT, M], FP8, tag="kT", name="kTn_sb")
                    wk_next = layer_w[l + 1][1]
                    for c in range(NCHUNK):
                        for j in range(DT):
                            kq.append(
                                lambda c=c, j=j, wkn=wk_next, ktn=kT_next:
                                emit_kproj_group(wkn, ktn, c, j))

                # last layer: gate x0-half as filler (pre-scaled x32 logits)
                if l == L - 1:
                    ws0c = sb.tile([128, DT, D], FP8, tag="wsc", bufs=2,
                                   name="ws0c_sb")
                    nc.sync.dma_start(
                        out=ws0c,
                        in_=ws_d[0:D, :].rearrange("(s p) n -> p s n", p=128))
                    # x0b = x0 - be2 (bf16), consumed by the tail blend
                    x0b = sb.tile([128, DT, R], BF16, tag="x0b", bufs=1,
                                  name="x0b_sb")
                    for a in range(DT):
                        nc.vector.tensor_scalar(
                            out=x0b[:, a, :], in0=x0[:, a, :],
                            scalar1=prm[:, base + P_BE2 + a:base + P_BE2 + a + 1],
                            scalar2=None, op0=ALU.subtract)
                    gstash = sb.tile([128, DT, R], BF16, tag="gstash", bufs=1,
                                     name="gstash_sb")

                    def emit_gate_x0(j):
                        gxp = ps.tile([128, R], F32, tag="z", bufs=1, name="gx_ps")
                        for ap_ in range(2):
                            nc.tensor.matmul(gxp, ws0c[:, 2 * ap_:2 * ap_ + 2, ds(j * 128, 128)],
                                             xin8[:, 2 * ap_:2 * ap_ + 2, :],
                                             start=(ap_ == 0), stop=(ap_ == 1),
                                             perf_mode=DR)
                        nc.vector.tensor_scalar(out=gstash[:, j, :], in0=gxp,
                                                scalar1=prm[:, P_BS + j:P_BS + j + 1],
                                                scalar2=None, op0=ALU.add)

                    for j in range(DT):
                        gateq.append(lambda j=j: emit_gate_x0(j))

                # --- LN1 ----------------------------------------------------
                xlnf8 = sb.tile([128, DT, R], FP8, tag="xlnf8", bufs=2, name="xln_sb")
                layernorm(x, base + P_G1, base + P_BE1, xlnf8)

                # --- FFN (fp8 DR, fused pipeline) ---------------------------
                f2ps = ps.tile([128, DT, R], F32, tag="acc4", bufs=1, name="f2_ps")
                w1c, w2c = {}, {}

                def load_ffn_chunk(og, l=l):
                    w1c[og] = sb.tile([128, DT, 512], FP8, tag="wc", bufs=4,
                                      name="w1c_sb")
                    nc.sync.dma_start(
                        out=w1c[og],
                        in_=w1_d[l][:, ds(og * 512, 512)].rearrange(
                            "(a p) n -> p a n", p=128))
                    w2c[og] = sb.tile([128, DT, 512], FP8, tag="wc", bufs=4,
                                      name="w2c_sb")
                    nc.sync.dma_start(
                        out=w2c[og],
                        in_=w2_d[l][ds(og * 512, 512), :].rearrange(
                            "(s p) n -> p s n", p=128))

                load_ffn_chunk(0)

                def emit_f2(h2, op):
                    for j in range(DT):
                        nc.tensor.matmul(
                            f2ps[:, j, :],
                            w2c[op // 2][:, (2 * op) % 4:(2 * op) % 4 + 2, ds(j * 128, 128)],
                            h2, start=(op == 0), stop=(op == FT // 2 - 1),
                            perf_mode=DR, skip_group_check=True)

                hq = []
                h2 = None
                for o in range(FT):
                    fps = ps.tile([128, R], F32, tag="mm", bufs=3, name="f1_ps")
                    for ap_ in range(2):
                        nc.tensor.matmul(
                            fps,
                            w1c[o // 4][:, 2 * ap_:2 * ap_ + 2, ds((o % 4) * 128, 128)],
                            xlnf8[:, 2 * ap_:2 * ap_ + 2, :],
                            start=(ap_ == 0), stop=(ap_ == 1), perf_mode=DR)
                    if o % 2 == 0:
                        h2 = sb.tile([128, 2, R], FP8, tag="h", bufs=3, name="h_sb")
                    nc.scalar.activation(out=h2[:, o % 2, :], in_=fps, func=AF.Gelu,
                                         bias=prm[:, base + P_B1 + o:base + P_B1 + o + 1],
                                         scale=IVS)
                    if o % 2 == 1:
                        hq.append((h2, o // 2))
                        if len(hq) > 1:
                            emit_f2(*hq.pop(0))
                    if o % 2 == 0:
                        drain_filler(1)
                    if o % 4 == 3 and o // 4 + 1 < 4:
                        load_ffn_chunk(o // 4 + 1)
                for h_o in hq:
                    emit_f2(*h_o)

                # FFN residual (x32 unscale + b2, then add)
                for j in range(DT):
                    t = sb.tile([128, R], BF16, tag="fr", bufs=2, name="fr_sb")
                    nc.vector.tensor_scalar(out=t, in0=f2ps[:, j, :], scalar1=IVS,
                                            scalar2=prm[:, base + P_B2 + j:base + P_B2 + j + 1],
                                            op0=ALU.mult, op1=ALU.add)
                    nc.vector.tensor_add(x[:, j, :], x[:, j, :], t)
                    drain_filler(1)
                    if j % 2:
                        warm_tick()

                if l < L - 1:
                    # --- LN2 -> next layer input --------------------------
                    xf8_next = sb.tile([128, DT, R], FP8, tag="xf8n", bufs=1,
                                       name="xf8n_sb")
                    layernorm(x, base + P_G2, base + P_BE2, xf8_next)
                    xf8_cur = xf8_next
                else:
                    # =========== tail: LN2 + gate + blend + output ========
                    ws1c = sb.tile([128, DT, D], FP8, tag="wsc", bufs=2,
                                   name="ws1c_sb")
                    nc.sync.dma_start(
                        out=ws1c,
                        in_=ws_d[D:2 * D, :].rearrange("(s p) n -> p s n", p=128))
                    ensure_k(32)
                    drain_filler(len(gateq))
                    # raw-x cast for the gate matmuls (ScalarE, off DVE)
                    xp8 = sb.tile([128, DT, R], FP8, tag="xp8", bufs=1, name="xp8_sb")
                    for a in range(DT):
                        nc.scalar.activation(out=xp8[:, a, :], in_=x[:, a, :],
                                             func=AF.Identity)
                    mups, sqps = _ln_stats(x)
                    # gate x-half matmuls on raw x, during the stats chain
                    gps = ps.tile([128, DT, R], F32, tag="acc4", bufs=1, name="g_ps")
                    for j in range(DT):
                        for ap_ in range(2):
                            nc.tensor.matmul(
                                gps[:, j, :],
                                ws1c[:, 2 * ap_:2 * ap_ + 2, ds(j * 128, 128)],
                                xp8[:, 2 * ap_:2 * ap_ + 2, :],
                                start=(ap_ == 0), stop=(ap_ == 1), perf_mode=DR)
                    rstd32, rstdb, murstd = _ln_scalar_chain(mups, sqps)
                    # blend: out = x0 + (1-g)*(xln - x0); (1-g) via negated
                    # sigmoid, xln - x0 = u*g2 - (x0 - be2)  [x0b, bf16]
                    for a in range(DT):
                        t = sb.tile([128, R], BF16, tag="lnt", bufs=2, name="ln_t2")
                        nc.vector.tensor_mul(t, x[:, a, :], rstdb)
                        u = sb.tile([128, R], BF16, tag="lnu", bufs=4, name="ln_u2")
                        nc.vector.tensor_sub(u, t, murstd)
                        d = sb.tile([128, R], BF16, tag="gt", bufs=3, name="d_sb")
                        nc.vector.scalar_tensor_tensor(
                            out=d, in0=u, scalar=prm[:, base + P_G2 + a:base + P_G2 + a + 1],
                            in1=x0b[:, a, :], op0=ALU.mult, op1=ALU.subtract)
                        # gate logit: rstd*(ws1g@x) - mu*rstd*S1 (+x0 half)
                        t1 = sb.tile([128, R], BF16, tag="gt", bufs=3, name="t1_sb")
                        nc.vector.tensor_mul(t1, gps[:, a, :], rstd32)
                        lg = sb.tile([128, R], BF16, tag="gt", bufs=3, name="lg_sb")
                        nc.vector.scalar_tensor_tensor(
                            out=lg, in0=murstd, scalar=prm[:, P_S1 + a:P_S1 + a + 1],
                            in1=t1, op0=ALU.mult, op1=ALU.add)
                        lg2 = sb.tile([128, R], BF16, tag="gt", bufs=3, name="lg2_sb")
                        nc.vector.tensor_add(lg2, lg, gstash[:, a, :])
                        wbar = sb.tile([128, R], BF16, tag="gt", bufs=3, name="wb_sb")
                        nc.scalar.activation(out=wbar, in_=lg2, func=AF.Sigmoid,
                                             bias=prm[:, P_C + a:P_C + a + 1],
                                             scale=-IVS)
                        m2 = sb.tile([128, R], BF16, tag="gt", bufs=3, name="m2_sb")
                        nc.vector.tensor_mul(m2, wbar, d)
                        ov = sb.tile([128, R], F32, tag="ov", bufs=2, name="ov_sb")
                        nc.vector.tensor_add(ov, x0[:, a, :], m2)
                        nc.sync.dma_start(out=out_d[a * 128:(a + 1) * 128, :], in_=ov)
                        warm_tick()

    nc.compile()
    return nc


_NC = None


def _get_nc():
    global _NC
    if _NC is None:
        _NC = _build()
    return _NC


def _core_items(d):
    return np.arange(d, B, NCORES)


def _make_in_maps(inputs):
    cog = np.asarray(inputs["cognition_features"], np.float32)
    flat = cog.reshape(M, D)
    cogT = np.ascontiguousarray(flat.T)          # [D, M] fp32

    common = {
        "memT8": cogT.astype(F8NP),
        "mem8": flat.astype(F8NP),
    }
    for l in range(L):
        common[f"wq{l}"] = np.ascontiguousarray(np.asarray(inputs["Wq"][l], np.float32).T * WS).astype(F8NP)
        common[f"wk{l}"] = np.ascontiguousarray(np.asarray(inputs["Wk"][l], np.float32).T * WS).astype(F8NP)
        common[f"wv{l}"] = np.ascontiguousarray(np.asarray(inputs["Wv"][l], np.float32).T * WS).astype(F8NP)
        common[f"w1{l}"] = np.ascontiguousarray(np.asarray(inputs["W1"][l], np.float32).T * WS).astype(F8NP)
        common[f"w2{l}"] = np.ascontiguousarray(np.asarray(inputs["W2"][l], np.float32).T * WS).astype(F8NP)

    # gate weights: [2D, D] = Ws.T; attn-half rows folded with ln2 gain, x32
    wsT = np.ascontiguousarray(np.asarray(inputs["Ws"], np.float32).T)  # [2D, D]
    g2 = np.asarray(inputs["ln2_g"][L - 1], np.float32)
    be2 = np.asarray(inputs["ln2_b"][L - 1], np.float32)
    ws_scaled = wsT.copy()
    ws_scaled[:D, :] *= WS
    ws_scaled[D:, :] = wsT[D:, :] * g2[:, None] * WS
    common["ws"] = ws_scaled.astype(F8NP)
    S1 = (wsT[D:, :] * g2[:, None]).sum(axis=0)          # [D]
    Cc = (wsT[D:, :] * be2[:, None]).sum(axis=0)         # [D]

    prm = np.zeros((128, P_COLS), np.float32)

    def put(col, vec):
        v = np.asarray(vec, np.float32).reshape(-1, 128)
        for j in range(v.shape[0]):
            prm[:, col + j] = v[j]

    for l in range(L):
        base = 48 * l
        put(base + P_BQ, inputs["bq"][l])
        put(base + P_BV, inputs["bv"][l])
        put(base + P_G1, inputs["ln1_g"][l])
        put(base + P_BE1, inputs["ln1_b"][l])
        put(base + P_B1, inputs["b1"][l])
        put(base + P_B2, inputs["b2"][l])
        put(base + P_G2, inputs["ln2_g"][l])
        put(base + P_BE2, inputs["ln2_b"][l])
    put(P_BS, np.asarray(inputs["bs"], np.float32) * WS)
    put(P_S1, -WS * S1)
    put(P_C, -Cc)   # negated: tail computes (1-g) = sigmoid(-logit)
    common["params"] = prm

    item_of_m = (np.arange(M) // T).astype(np.float32)
    common["item_vals"] = np.ascontiguousarray(
        item_of_m.reshape(MT, 128).T).astype(BF)

    in_maps = []
    for d in range(NCORES):
        items = _core_items(d)
        rows = cog[items].reshape(R, D)          # [R, D] this core's rows
        rT = np.ascontiguousarray(rows.T)        # [D, R]
        b_of_r = np.repeat(items, T).astype(np.float32)
        im = dict(common)
        im["xin8"] = rT.astype(F8NP)
        im["x_bf"] = rT.astype(BF)
        im["x0_f32"] = rT
        im["b_bcast"] = np.broadcast_to(b_of_r, (128, R)).astype(BF).copy()
        in_maps.append(im)
    return in_maps


def _run(in_maps, trace=False):
    nc = _get_nc()
    return run_bass_kernel_spmd(nc, in_maps, list(range(NCORES)), trace=trace)


def kernel(**inputs):
    in_maps = _make_in_maps(inputs)
    res = _run(in_maps)
    cog = np.asarray(inputs["cognition_features"], np.float32)
    out = np.empty((B, T, D), np.float32)
    for d in range(NCORES):
        out[_core_items(d)] = res.results[d]["outT"].T.reshape(BLOC, T, D)
    out[0] = cog[0]          # item 0: gate blends x0 with x0 -> exactly x0
    return out


if __name__ == "__main__":
    _build()
    print("build ok")
